# revision 9
# baseline (speedup 1.0000x reference)
"""Trainium2 Bass kernel for BertSelfAttention variant (logsigmoid-fused QK attention).

Reference computation (B=2, S=2048, D=1024, H=16, dh=64):
    q = split_heads(hidden @ Wq + bq)
    k = split_heads(hidden @ Wk + bk)
    k1 = logsigmoid(q) + q + k ; k2 = logsigmoid(k1)
    scores = -(q @ k2^T) / 8 ; probs = softmax(scores) ; ctx = probs @ q

With kk := -k2 = ln(1 + e^{-q-k} + e^{-2q-k}) = ln(1 + e^{-q-k}(1 + e^{-q})),
scores == +(q . kk)/8.  Only two projection chains are needed (Wq and Wq+Wk,
host-fused): eu = e^{-(q+k)} from the fused chain, eq = e^{-q} from the q
chain, kst = (eq+1)*eu on DVE, kk = Ln(kst+1).

Sharding: 8 cores = 2 (batch) x 4 (head groups of 4 heads / 256 cols of Wq,Wk).
Each core computes its [2048, 256] slice of the output; host reassembles.

Device layout is fully transposed so no matmul ever needs a transposed operand:
    qT, kkT [256(dout), 2048(s)]  from  ht = hidden[b].T  (host-side transpose)
    scoresT[kpos, q] = kkT_head^T @ qT_head   (row-tiled head pairs, K=64,
        tile_position (0,0)/(64,0) -> the two matmuls run concurrently)
    expT = Exp(scoresT / 8)                   (one [128,1024] ACT op per chunk)
    ctx[dh, q] = sum_kc v[kpos,64]^T @ expT   (col-tiled head pairs,
        tile_position (0,0)/(0,64) -> concurrent, separate PSUM banks)
    den[q] = ones^T @ (sum_kc expT)           (the kc-sum runs on DVE+GpSimd,
        so the PE only does one 128-deep reduction per stream)
Raw transposed ctx + denominators go back to DRAM; the host divides and
re-transposes while unsharding.

Schedule: software pipeline over 8 t-major q-streams; stream i's exps overlap
stream i-1's ctx drain.  Projection chunks, v transposes and the second-half
Ln ride in the early streams' spare PE slots.  One 8-bank PSUM pool:
sp(2x2 banks) qp ap cd(2) - ctx/den share cd, the last stream's immediate
drain reuses qp/ap after projections finish.  A single combined exp+ln
activation table avoids mid-kernel ACT_TABLE_LOAD switches.

Matmuls run in bf16; accumulators and the output stay fp32.
"""

import numpy as np

B, S, D = 2, 2048, 1024
H, DH = 16, 64
NCORES = 8
HG = 4  # head-group count (tensor parallel)
CPG = (H // HG) * DH  # cols per group = 256
NDT = D // 128  # 8 din tiles
NSC = S // 512  # 4 s-chunks (projection) == 4 q-chunks (attention)
NKC = S // 128  # 16 kpos chunks

_compiled = None
LAST_RESULT = None


def _build():
    from contextlib import ExitStack

    import concourse.bacc as bacc
    import concourse.mybir as mybir
    import concourse.tile as tile

    # Use the one activation table that holds BOTH Exp and Ln, so the kernel
    # does a single ACT_TABLE_LOAD instead of switching tables around each Ln.
    _orig_get_tables = bacc.get_activation_tables

    def _only_combined(arch):
        t = _orig_get_tables(arch)
        name = "natural_log_exp_and_others"
        if name not in t:
            return t
        # act_func_set_id is the INDEX into act_info.json's act_func_sets, so
        # keep every entry (and its position) but empty the others' function
        # sets so the load-insertion pass can only ever pick the combined one.
        return {k: (v if k == name else set()) for k, v in t.items()}

    bacc.get_activation_tables = _only_combined

    f32 = mybir.dt.float32
    mmdt = mybir.dt.bfloat16
    AF = mybir.ActivationFunctionType
    ALU = mybir.AluOpType

    nc = bacc.Bacc("TRN2", target_bir_lowering=False, debug=False)
    ht = nc.dram_tensor("ht", [D, S], mmdt, kind="ExternalInput").ap()
    # host-fused weight wall: [Wq | Wq+Wk] per row
    wall = nc.dram_tensor("wall", [D, 2 * CPG], mmdt, kind="ExternalInput").ap()
    # smalls cols: pbq[0:2] nbq[2:4] nbqk[4:6]
    smalls = nc.dram_tensor("smalls", [128, 6], f32, kind="ExternalInput").ap()
    # identity for the v transposes (I64 stacked twice) and a ones column
    idb = nc.dram_tensor("idb", [128, 65], mmdt, kind="ExternalInput").ap()
    out = nc.dram_tensor("out", [CPG, S], f32, kind="ExternalOutput").ap()
    dens = nc.dram_tensor("dens", [2 * NSC, 1024], f32, kind="ExternalOutput").ap()

    with tile.TileContext(nc) as tc, ExitStack() as ctx:
        const = ctx.enter_context(tc.tile_pool(name="const", bufs=1))
        big = ctx.enter_context(tc.tile_pool(name="big", bufs=1))
        sb = ctx.enter_context(tc.tile_pool(name="sb", bufs=4))
        # PSUM, exactly 8 banks: sp (2 bufs x [128,1024] = 4 banks),
        # qp, ap (1 bank each, projections), cd (2 bufs x 1 bank: ctx + den).
        # The last stream's immediate ctx + tpv transposes reuse qp/ap.
        ps = ctx.enter_context(tc.tile_pool(name="ps", bufs=1, space="PSUM"))
        etp = ctx.enter_context(tc.tile_pool(name="etp", bufs=32))
        accp = ctx.enter_context(tc.tile_pool(name="accp", bufs=2))
        csp = ctx.enter_context(tc.tile_pool(name="csp", bufs=4))

        sm = const.tile([128, 6], f32, tag="smalls")
        nc.sync.dma_start(sm[:], smalls[:])
        pbq_t = sm[:, 0:2]
        nbq_t = sm[:, 2:4]
        nbqk_t = sm[:, 4:6]
        idb_t = const.tile([128, 65], mmdt, tag="idb")
        nc.sync.dma_start(idb_t[:], idb[:])
        ones_t = idb_t[:, 64:65]

        # Weights + the sc0 hidden chunk first (they gate the first projection
        # chain), interleaved across both HWDGE rings; later hidden chunks
        # follow in sc order so proj(t, sc) unblocks progressively.
        rr_ring = [nc.sync, nc.scalar]
        ring_i = 0

        def ring():
            nonlocal ring_i
            ring_i += 1
            return rr_ring[ring_i % 2]

        wqs, was, hts = [], [], []
        for j in range(NDT):
            t_ = big.tile([128, S], mmdt, tag=f"ht{j}", name=f"hts{j}")
            hts.append(t_)
        for j in range(NDT):
            w = const.tile([128, 2 * CPG], mmdt, tag=f"w{j}", name=f"walls{j}")
            ring().dma_start(w[:], wall[j * 128 : (j + 1) * 128, :])
            wqs.append(w[:, 0:CPG])
            was.append(w[:, CPG : 2 * CPG])
            ring().dma_start(hts[j][:, 0:512], ht[j * 128 : (j + 1) * 128, 0:512])
        for sc in range(1, NSC):
            ssl = slice(sc * 512, (sc + 1) * 512)
            for j in range(NDT):
                nc.sync.dma_start(hts[j][:, ssl], ht[j * 128 : (j + 1) * 128, ssl])

        q_sb = [big.tile([128, S], mmdt, tag=f"q{t}", name=f"q{t}") for t in range(2)]
        kk_sb = [big.tile([128, S], mmdt, tag=f"kk{t}", name=f"kk{t}") for t in range(2)]
        kst = [big.tile([128, S], f32, tag=f"kst{t}", name=f"kst{t}") for t in range(2)]
        vaug = [
            big.tile([128, NKC * 64], mmdt, tag=f"v{h}", name=f"v{h}") for h in range(4)
        ]

        # ---------- emission helpers ----------
        def emit_proj_chunk(t, sc, ln=True):
            """Two matmul chains for one [dout-half, 512] chunk + elementwise."""
            ssl = slice(sc * 512, (sc + 1) * 512)
            qp = ps.tile([128, 512], f32, tag="qp", name="qp", bufs=1)
            for j in range(NDT):
                nc.tensor.matmul(
                    qp[:],
                    lhsT=wqs[j][:, t * 128 : (t + 1) * 128],
                    rhs=hts[j][:, ssl],
                    start=(j == 0),
                    stop=(j == NDT - 1),
                )
            ap = ps.tile([128, 512], f32, tag="ap", name="ap", bufs=1)
            for j in range(NDT):
                nc.tensor.matmul(
                    ap[:],
                    lhsT=was[j][:, t * 128 : (t + 1) * 128],
                    rhs=hts[j][:, ssl],
                    start=(j == 0),
                    stop=(j == NDT - 1),
                )
            eq = sb.tile([128, 512], f32, tag="eq")
            nc.scalar.activation(
                eq[:], qp[:], AF.Exp, bias=nbq_t[:, t : t + 1], scale=-1.0
            )
            nc.vector.tensor_scalar_add(q_sb[t][:, ssl], qp[:], pbq_t[:, t : t + 1])
            eu = sb.tile([128, 512], f32, tag="eu")
            nc.scalar.activation(
                eu[:], ap[:], AF.Exp, bias=nbqk_t[:, t : t + 1], scale=-1.0
            )
            # kst = (eq + 1) * eu = e^{-q-k} + e^{-2q-k}
            nc.vector.scalar_tensor_tensor(
                kst[t][:, ssl], eq[:], 1.0, eu[:], ALU.add, ALU.mult
            )
            if ln:
                nc.scalar.activation(
                    kk_sb[t][:, ssl], kst[t][:, ssl], AF.Ln, bias=1.0, scale=1.0
                )

        def emit_ln(t):
            nc.scalar.activation(kk_sb[t][:], kst[t][:], AF.Ln, bias=1.0, scale=1.0)

        def ps_tag_bufs(tag):
            return 2 if tag == "cd" else 1

        def emit_vaug_chunk(t, j, tags):
            """PE-transpose one [64,128] q chunk per head of half t."""
            for rr in range(2):
                lh = 2 * t + rr
                hsl = slice(rr * 64, rr * 64 + 64)
                # sized [128,1024] bf16 = 2KB so the tag region matches the
                # [128,512] f32 ctx/den/proj tiles that share these banks
                tpv = ps.tile(
                    [128, 1024], mmdt, tag=tags[rr], name="tpv",
                    bufs=ps_tag_bufs(tags[rr]),
                )
                nc.tensor.transpose(
                    tpv[:, 0:64], q_sb[t][hsl, j * 128 : (j + 1) * 128], idb_t[hsl, 0:64]
                )
                nc.vector.tensor_copy(vaug[lh][:, j * 64 : j * 64 + 64], tpv[:, 0:64])

        def run_filler(item):
            if item[0] == "vaug":
                for j in item[2]:
                    emit_vaug_chunk(item[1], j, item[3])
            elif item[0] == "proj":
                emit_proj_chunk(item[1], item[2], ln=(item[1] == 0))
            elif item[0] == "ln":
                emit_ln(item[1])

        def emit_den_acc(st, kc, et):
            """Accumulate et into the stream's denominator partials.

            Even kc chunks go to accA on DVE, odd to accB on GpSimd, halving
            the per-engine cost; the two merge at the next stream's start.
            """
            accA, accB = st["accs"]
            eng = nc.vector if kc % 2 == 0 else nc.gpsimd
            acc = accA if kc % 2 == 0 else accB
            if kc < 2:
                eng.tensor_copy(acc[:], et[:])
            else:
                eng.tensor_add(acc[:], acc[:], et[:])

        def emit_den_finish(st):
            """accb = accA+accB (bf16), then den = ones^T @ accb on the PE."""
            accA, accB = st["accs"]
            accb = accp.tile([128, 1024], mmdt, tag="accb", name="accb")
            nc.vector.tensor_add(accb[:], accA[:], accB[:])
            st["accb"] = accb

        def emit_den_mm_copy(st, tag):
            accb = st["accb"]
            dps = ps.tile([128, 512], f32, tag=tag, name="denps", bufs=ps_tag_bufs(tag))
            nc.tensor.matmul(
                dps[0:1, :], lhsT=ones_t, rhs=accb[:, 0:512], start=True, stop=True
            )
            nc.tensor.matmul(
                dps[32:33, :], lhsT=ones_t, rhs=accb[:, 512:1024], start=True, stop=True
            )
            db = csp.tile([33, 512], f32, tag="densb", name="densb")
            nc.vector.tensor_copy(db[:], dps[0:33, :])
            i = st["i"]
            nc.sync.dma_start(dens[i : i + 1, 0:512], db[0:1, :])
            nc.sync.dma_start(dens[i : i + 1, 512:1024], db[32:33, :])

        def emit_drain_pair(st, j, ctag):
            """Col-tiled concurrent ctx accumulation: h0 into ctxs[0][0:64]
            (tile_position (0,0)), h1 into ctxs[1][64:128] ((0,64))."""
            qsl = st["qsl"]
            t = st["t"]
            ets = st["ets"]
            for rr in range(2):
                if j[rr] is None:
                    continue
                jj = j[rr]
                if st["ctxs"][rr] is None:
                    st["ctxs"][rr] = ps.tile(
                        [128, 512], f32, tag=ctag[rr], name=f"ctx{rr}",
                        bufs=ps_tag_bufs(ctag[rr]),
                    )
                nc.tensor.matmul(
                    st["ctxs"][rr][rr * 64 : rr * 64 + 64, :],
                    lhsT=vaug[2 * t + rr][:, jj * 64 : jj * 64 + 64],
                    rhs=ets[jj][:, rr * 512 : rr * 512 + 512],
                    start=(jj == st["start_j"]),
                    stop=(jj == st["stop_j"]),
                )

        def emit_finalize(st):
            qsl = st["qsl"]
            t = st["t"]
            cs = csp.tile([128, 512], f32, tag="cs")
            nc.vector.tensor_copy(cs[0:64, :], st["ctxs"][0][0:64, :])
            nc.vector.tensor_copy(cs[64:128, :], st["ctxs"][1][64:128, :])
            nc.sync.dma_start(out[t * 128 : t * 128 + 128, qsl], cs[:])

        # ---------- schedule ----------
        emit_proj_chunk(0, 0)

        streams = [(qc, t) for t in range(2) for qc in range(NSC)]
        filler = {
            0: [
                ("proj", 0, 1),
                ("vaug", 0, range(0, 4), ("cd", "cd")),
                ("proj", 0, 2),
                ("vaug", 0, range(4, 8), ("cd", "cd")),
                ("proj", 0, 3),
                ("vaug", 0, range(8, 12), ("cd", "cd")),
                ("vaug", 0, range(12, 16), ("cd", "cd")),
                ("proj", 1, 0),
            ],
            1: [
                ("proj", 1, 1),
                ("proj", 1, 2),
                ("proj", 1, 3),
                ("ln", 1),
            ],
            2: [
                ("vaug", 1, range(0, 4), ("qp", "ap")),
                ("vaug", 1, range(4, 8), ("qp", "ap")),
            ],
            3: [
                ("vaug", 1, range(8, 12), ("qp", "ap")),
                ("vaug", 1, range(12, 16), ("qp", "ap")),
            ],
        }

        prev = None
        for i, (qc, t) in enumerate(streams):
            qsl = slice(qc * 512, (qc + 1) * 512)
            fill = list(filler.get(i, []))
            last = i == len(streams) - 1
            accA = accp.tile([128, 1024], f32, tag="accA", name="accA")
            accB = accp.tile([128, 1024], f32, tag="accB", name="accB")
            st = {
                "i": i,
                "qc": qc,
                "t": t,
                "qsl": qsl,
                "ets": [],
                "accs": (accA, accB),
                "accb": None,
                "ctxs": [None, None],
                # immediate (last stream) drains ascending, prev drains descending
                "start_j": 0 if last else NKC - 1,
                "stop_j": NKC - 1 if last else 0,
            }
            if prev is not None:
                emit_den_finish(prev)
            for kc in range(NKC):
                ksl = slice(kc * 128, (kc + 1) * 128)
                sp = ps.tile([128, 1024], f32, tag="sp", name="sp", bufs=2)
                nc.tensor.matmul(
                    sp[:, 0:512],
                    lhsT=kk_sb[t][0:64, ksl],
                    rhs=q_sb[t][0:64, qsl],
                    start=True,
                    stop=True,
                )
                nc.tensor.matmul(
                    sp[:, 512:1024],
                    lhsT=kk_sb[t][64:128, ksl],
                    rhs=q_sb[t][64:128, qsl],
                    start=True,
                    stop=True,
                )
                et = etp.tile([128, 1024], mmdt, tag="et", name=f"et{kc}")
                nc.scalar.activation(et[:], sp[:], AF.Exp, scale=0.125)
                st["ets"].append(et)
                emit_den_acc(st, kc, et)
                if prev is not None:
                    # staggered: kc=0 -> h0_15 solo; kc>=1 -> (h1_{16-kc}, h0_{15-kc})
                    j0 = NKC - 1 - kc
                    j1 = None if kc == 0 else NKC - kc
                    emit_drain_pair(prev, (j0, j1), ("cd", "cd"))
                if last:
                    # immediate: kc=0 -> h0_0 solo; kc>=1 -> (h0_kc, h1_{kc-1})
                    j0 = kc
                    j1 = None if kc == 0 else kc - 1
                    emit_drain_pair(st, (j0, j1), ("qp", "ap"))
                if fill and (kc % 2 == 1 or len(fill) >= NKC - kc):
                    run_filler(fill.pop(0))
            for item in fill:
                run_filler(item)
            if prev is not None:
                emit_drain_pair(prev, (None, 0), ("cd", "cd"))  # h1_0
                emit_finalize(prev)
                emit_den_mm_copy(prev, "cd")
            if last:
                emit_drain_pair(st, (None, NKC - 1), ("qp", "ap"))  # h1_15
                emit_den_finish(st)
                emit_finalize(st)
                emit_den_mm_copy(st, "qp")
                prev = None
            else:
                prev = st

    nc.compile()
    bacc.get_activation_tables = _orig_get_tables
    return nc


def kernel(hidden_states, attention_mask, Wq, bq, Wk, bk):
    global _compiled, LAST_RESULT
    hs = np.asarray(hidden_states, dtype=np.float32)
    am = np.asarray(attention_mask)
    Wq = np.asarray(Wq, dtype=np.float32)
    Wk = np.asarray(Wk, dtype=np.float32)
    bq = np.asarray(bq, dtype=np.float32)
    bk = np.asarray(bk, dtype=np.float32)

    if _compiled is None:
        _compiled = _build()
    nc = _compiled

    from concourse.bass_utils import run_bass_kernel_spmd

    import ml_dtypes

    def to_mmdt(x):
        return np.ascontiguousarray(np.asarray(x, np.float32).astype(ml_dtypes.bfloat16))

    idb = to_mmdt(
        np.concatenate(
            [np.tile(np.eye(64, dtype=np.float32), (2, 1)), np.ones((128, 1))], axis=1
        )
    )
    in_maps = []
    for c in range(NCORES):
        b, g = c // HG, c % HG
        cols = slice(g * CPG, (g + 1) * CPG)
        bq_s = bq[cols].reshape(2, 128).T
        bk_s = bk[cols].reshape(2, 128).T
        smalls = np.concatenate(
            [bq_s, -bq_s, -(bq_s + bk_s)], axis=1
        ).astype(np.float32)
        in_maps.append(
            {
                "ht": to_mmdt(hs[b].T),
                "wall": to_mmdt(
                    np.concatenate([Wq[:, cols], Wq[:, cols] + Wk[:, cols]], axis=1)
                ),
                "smalls": np.ascontiguousarray(smalls),
                "idb": idb,
            }
        )

    res = run_bass_kernel_spmd(nc, in_maps, list(range(NCORES)))
    LAST_RESULT = res

    outp = np.empty((B, S, H * DH), dtype=np.float32)
    for c in range(NCORES):
        b, g = c // HG, c % HG
        ctxT = res.results[c]["out"]  # [256, 2048] raw ctx sums (transposed)
        dn = res.results[c]["dens"]  # [8, 1024]: row t*4+qc = [h0 512q | h1 512q]
        den = np.empty((4, S), dtype=np.float32)
        for t in range(2):
            for qc in range(NSC):
                for rr in range(2):
                    den[t * 2 + rr, qc * 512 : (qc + 1) * 512] = dn[
                        t * NSC + qc, rr * 512 : (rr + 1) * 512
                    ]
        ctxT = ctxT.reshape(4, 64, S) / den[:, None, :]
        outp[b, :, g * CPG : (g + 1) * CPG] = ctxT.reshape(CPG, S).T

    # attention_mask==0 masks whole query rows -> uniform probs -> ctx row is
    # the mean of q over all key positions. Never triggers for all-ones masks.
    if (am == 0).any():
        for b in range(B):
            rows = np.nonzero(am[b] == 0)[0]
            if rows.size:
                q_full = hs[b] @ Wq + bq
                outp[b, rows, :] = q_full.mean(axis=0)
    return outp


# revision 12
# speedup vs baseline: 1.3173x; 1.3173x over previous
"""Trainium2 Bass kernel for BertSelfAttention variant (logsigmoid-fused QK attention).

Reference computation (B=2, S=2048, D=1024, H=16, dh=64):
    q = split_heads(hidden @ Wq + bq)
    k = split_heads(hidden @ Wk + bk)
    k1 = logsigmoid(q) + q + k ; k2 = logsigmoid(k1)
    scores = -(q @ k2^T) / 8 ; probs = softmax(scores) ; ctx = probs @ q

With kk := -k2 = ln(1 + e^{-q-k} + e^{-2q-k}) = ln(1 + e^{-q-k}(1 + e^{-q})),
scores == +(q . kk)/8.  Only two projection chains are needed (Wq and Wq+Wk,
host-fused): eu = e^{-(q+k)} from the fused chain, eq = e^{-q} from the q
chain (both straight from PSUM), kst = (eq+1)*eu in one DVE op, kk = Ln(kst+1).

Sharding: 8 cores = 2 (batch) x 4 (head groups of 4 heads / 256 cols of Wq,Wk).
Each core computes its [2048, 256] slice of the output; host reassembles.

Device layout is fully transposed so no matmul ever needs a transposed operand:
    qT, kkT [256(dout), 2048(s)]  from  ht = hidden[b].T  (host-side transpose)
    scoresT[kpos, q] = kkT_head^T @ qT_head   (row-tiled head pairs, K=64 at
        tile positions (0,0)/(64,0) -> the two matmuls run concurrently)
    expT = Exp(scoresT / 8)                   (one [128,1024] ACT op per chunk)
    ctx_aug[65, q] = sum_kpos v_aug[kpos,65]^T @ expT[kpos, q]
        v_aug = [v | 1] -> row 64 accumulates the softmax denominator.
Raw transposed ctx + denominators go back to DRAM; the host divides and
re-transposes while unsharding (no device-side finalize transposes).

Schedule: software pipeline over 8 t-major q-streams, per-chunk pipelined
startup (stream 0 begins after just the first projection chunk).  While ACT
streams the exps of stream i, the PE drains stream i-1's ctx accumulation
(drain pair emitted before the scores pair in each slot, so the PE never
head-blocks on a PSUM buffer with ready work behind it).  Projection chunks,
v transposes and the second-half Ln ride in the early streams' spare PE slots.
One 8-bank PSUM pool: sp(2x[128,1024]) qp ap cd(2x[65,512] ctx) - the tpv
transposes borrow cd (stream 0) / qp+ap (streams 2-3, after projections), and
the last stream's immediate ctx reuses qp/ap.  A single combined exp+ln
activation table avoids mid-kernel ACT_TABLE_LOAD switches.  Input DMA fans
out over four engine rings so the first projection chunk lands in ~6us.

Matmuls run in bf16; accumulators and the output stay fp32.
"""

import numpy as np

B, S, D = 2, 2048, 1024
H, DH = 16, 64
NCORES = 8
HG = 4  # head-group count (tensor parallel)
CPG = (H // HG) * DH  # cols per group = 256
NDT = D // 128  # 8 din tiles
NSC = S // 512  # 4 s-chunks (projection) == 4 q-chunks (attention)
NKC = S // 128  # 16 kpos chunks

_compiled = None
LAST_RESULT = None


def _build():
    from contextlib import ExitStack

    import concourse.bacc as bacc
    import concourse.mybir as mybir
    import concourse.tile as tile

    # Use the one activation table that holds BOTH Exp and Ln, so the kernel
    # does a single ACT_TABLE_LOAD instead of switching tables around each Ln.
    # act_func_set_id is the INDEX into act_info.json's act_func_sets, so keep
    # every entry (and its position) but empty the other sets so the
    # load-insertion pass can only ever pick the combined one.
    _orig_get_tables = bacc.get_activation_tables

    def _only_combined(arch):
        t = _orig_get_tables(arch)
        name = "natural_log_exp_and_others"
        if name not in t:
            return t
        return {k: (v if k == name else set()) for k, v in t.items()}

    bacc.get_activation_tables = _only_combined

    f32 = mybir.dt.float32
    mmdt = mybir.dt.bfloat16
    AF = mybir.ActivationFunctionType
    ALU = mybir.AluOpType

    nc = bacc.Bacc("TRN2", target_bir_lowering=False, debug=False)
    ht = nc.dram_tensor("ht", [D, S], mmdt, kind="ExternalInput").ap()
    # host-fused weight wall: [Wq | Wq+Wk] per row
    wall = nc.dram_tensor("wall", [D, 2 * CPG], mmdt, kind="ExternalInput").ap()
    # smalls cols: pbq[0:2] nbq[2:4] nbqk[4:6] ones[6:22]
    smalls = nc.dram_tensor("smalls", [128, 22], f32, kind="ExternalInput").ap()
    # identity for the v transposes (I64 stacked twice)
    idb = nc.dram_tensor("idb", [128, 64], mmdt, kind="ExternalInput").ap()
    out = nc.dram_tensor("out", [CPG, S], f32, kind="ExternalOutput").ap()
    dens = nc.dram_tensor("dens", [16, S // 4], f32, kind="ExternalOutput").ap()

    with tile.TileContext(nc) as tc, ExitStack() as ctx:
        const = ctx.enter_context(tc.tile_pool(name="const", bufs=1))
        big = ctx.enter_context(tc.tile_pool(name="big", bufs=1))
        sb = ctx.enter_context(tc.tile_pool(name="sb", bufs=4))
        # PSUM, exactly 8 banks: sp (2 bufs x [128,1024] = 4 banks),
        # qp, ap (1 bank each, projections), cd (2 bufs x 1 bank, ctx tiles).
        ps = ctx.enter_context(tc.tile_pool(name="ps", bufs=1, space="PSUM"))
        etp = ctx.enter_context(tc.tile_pool(name="etp", bufs=24))
        csp = ctx.enter_context(tc.tile_pool(name="csp", bufs=4))

        sm = const.tile([128, 22], f32, tag="smalls")
        nc.sync.dma_start(sm[:], smalls[:])
        pbq_t = sm[:, 0:2]
        nbq_t = sm[:, 2:4]
        nbqk_t = sm[:, 4:6]
        ones_t = sm[:, 6:22]
        idb_t = const.tile([128, 64], mmdt, tag="idb")
        nc.sync.dma_start(idb_t[:], idb[:])

        # Input DMA: weights + the sc0 hidden chunk gate the first projection
        # chunk, so interleave them across both HWDGE rings (SP + ACT, the
        # only engines that can initiate DMAs); together they run ~400GB/s.
        # Later hidden chunks follow in sc order so proj(t,sc) unblocks
        # progressively.
        rr_ring = [nc.sync, nc.scalar]
        ring_i = 0

        def ring():
            nonlocal ring_i
            ring_i += 1
            return rr_ring[ring_i % 2]

        wqs, was, hts = [], [], []
        for j in range(NDT):
            t_ = big.tile([128, S], mmdt, tag=f"ht{j}", name=f"hts{j}")
            hts.append(t_)
        for j in range(NDT):
            w = const.tile([128, 2 * CPG], mmdt, tag=f"w{j}", name=f"walls{j}")
            ring().dma_start(w[:], wall[j * 128 : (j + 1) * 128, :])
            wqs.append(w[:, 0:CPG])
            was.append(w[:, CPG : 2 * CPG])
            ring().dma_start(hts[j][:, 0:512], ht[j * 128 : (j + 1) * 128, 0:512])
        for sc in range(1, NSC):
            ssl = slice(sc * 512, (sc + 1) * 512)
            for j in range(NDT):
                nc.sync.dma_start(hts[j][:, ssl], ht[j * 128 : (j + 1) * 128, ssl])

        q_sb = [big.tile([128, S], mmdt, tag=f"q{t}", name=f"q{t}") for t in range(2)]
        kk_sb = [big.tile([128, S], mmdt, tag=f"kk{t}", name=f"kk{t}") for t in range(2)]
        kst = [big.tile([128, S], f32, tag=f"kst{t}", name=f"kst{t}") for t in range(2)]
        vaug = [
            big.tile([128, NKC * 65], mmdt, tag=f"v{h}", name=f"v{h}") for h in range(4)
        ]

        # ---------- emission helpers ----------
        def ps_tag_bufs(tag):
            return 2 if tag == "cd" else 1

        def emit_proj_chunk(t, sc, ln=True):
            """Two interleaved matmul chains for one [128, 512] chunk.

            Interleaving qp/ap per din tile means both chains finish one
            matmul after the last input DMA lands (startup critical path).
            """
            ssl = slice(sc * 512, (sc + 1) * 512)
            qp = ps.tile([128, 512], f32, tag="qp", name="qp", bufs=1)
            ap = ps.tile([128, 512], f32, tag="ap", name="ap", bufs=1)
            for j in range(NDT):
                nc.tensor.matmul(
                    qp[:],
                    lhsT=wqs[j][:, t * 128 : (t + 1) * 128],
                    rhs=hts[j][:, ssl],
                    start=(j == 0),
                    stop=(j == NDT - 1),
                )
                nc.tensor.matmul(
                    ap[:],
                    lhsT=was[j][:, t * 128 : (t + 1) * 128],
                    rhs=hts[j][:, ssl],
                    start=(j == 0),
                    stop=(j == NDT - 1),
                )
            eq = sb.tile([128, 512], f32, tag="eq")
            nc.scalar.activation(
                eq[:], qp[:], AF.Exp, bias=nbq_t[:, t : t + 1], scale=-1.0
            )
            nc.vector.tensor_scalar_add(q_sb[t][:, ssl], qp[:], pbq_t[:, t : t + 1])
            eu = sb.tile([128, 512], f32, tag="eu")
            nc.scalar.activation(
                eu[:], ap[:], AF.Exp, bias=nbqk_t[:, t : t + 1], scale=-1.0
            )
            # kst = (eq + 1) * eu = e^{-q-k} + e^{-2q-k}
            nc.vector.scalar_tensor_tensor(
                kst[t][:, ssl], eq[:], 1.0, eu[:], ALU.add, ALU.mult
            )
            if ln:
                nc.scalar.activation(
                    kk_sb[t][:, ssl], kst[t][:, ssl], AF.Ln, bias=1.0, scale=1.0
                )

        def emit_ln(t):
            nc.scalar.activation(kk_sb[t][:], kst[t][:], AF.Ln, bias=1.0, scale=1.0)

        def emit_vaug_ones(t):
            for rr in range(2):
                vv = vaug[2 * t + rr][:].rearrange("p (c w) -> p c w", w=65)
                nc.vector.tensor_copy(
                    vv[:, :, 64:65], ones_t.rearrange("p (c w) -> p c w", w=1)
                )

        def emit_vaug_chunk(t, j, tags):
            """PE-transpose one [64,128] q chunk per head of half t."""
            for rr in range(2):
                lh = 2 * t + rr
                hsl = slice(rr * 64, rr * 64 + 64)
                # sized [128,1024] bf16 = 2KB so the tag region matches the
                # [65,512]/[128,512] f32 tiles that share these banks
                tpv = ps.tile(
                    [128, 1024], mmdt, tag=tags[rr], name="tpv",
                    bufs=ps_tag_bufs(tags[rr]),
                )
                nc.tensor.transpose(
                    tpv[:, 0:64], q_sb[t][hsl, j * 128 : (j + 1) * 128], idb_t[hsl, 0:64]
                )
                nc.vector.tensor_copy(vaug[lh][:, j * 65 : j * 65 + 64], tpv[:, 0:64])

        def run_filler(item):
            if item[0] == "vaug":
                for j in item[2]:
                    emit_vaug_chunk(item[1], j, item[3])
            elif item[0] == "proj":
                emit_proj_chunk(item[1], item[2], ln=(item[1] == 0))
            elif item[0] == "ln":
                emit_ln(item[1])
            elif item[0] == "vones":
                emit_vaug_ones(item[1])

        def emit_drain_pair(st, jj):
            """Ctx accumulation for both heads of one kpos chunk (M=65)."""
            t = st["t"]
            ets = st["ets"]
            for rr in range(2):
                if st["ctxs"][rr] is None:
                    ctag = st["ctag"][rr]
                    st["ctxs"][rr] = ps.tile(
                        [65, 512], f32, tag=ctag, name=f"ctx{rr}",
                        bufs=ps_tag_bufs(ctag),
                    )
                nc.tensor.matmul(
                    st["ctxs"][rr][:],
                    lhsT=vaug[2 * t + rr][:, jj * 65 : jj * 65 + 65],
                    rhs=ets[jj][:, rr * 512 : rr * 512 + 512],
                    start=(jj == st["start_j"]),
                    stop=(jj == st["stop_j"]),
                )

        def emit_finalize(st):
            qsl = st["qsl"]
            qc, t = st["qc"], st["t"]
            for rr in range(2):
                cs = csp.tile([128, 512], f32, tag="cs")
                nc.vector.tensor_copy(cs[0:65, :], st["ctxs"][rr][:])
                lh = 2 * t + rr
                nc.sync.dma_start(out[lh * 64 : lh * 64 + 64, qsl], cs[0:64, :])
                r = qc * 4 + t * 2 + rr
                nc.sync.dma_start(dens[r : r + 1, :], cs[64:65, :])

        # ---------- schedule ----------
        emit_proj_chunk(0, 0)
        emit_vaug_ones(0)

        streams = [(qc, t) for t in range(2) for qc in range(NSC)]
        filler = {
            0: [
                ("proj", 0, 1),
                ("vaug", 0, range(0, 4), ("cd", "cd")),
                ("proj", 0, 2),
                ("vaug", 0, range(4, 8), ("cd", "cd")),
                ("proj", 0, 3),
                ("vaug", 0, range(8, 12), ("cd", "cd")),
                ("vaug", 0, range(12, 16), ("cd", "cd")),
                ("proj", 1, 0),
            ],
            1: [
                ("proj", 1, 1),
                ("proj", 1, 2),
                ("proj", 1, 3),
                ("ln", 1),
                ("vones", 1),
            ],
            2: [
                ("vaug", 1, range(0, 4), ("qp", "ap")),
                ("vaug", 1, range(4, 8), ("qp", "ap")),
            ],
            3: [
                ("vaug", 1, range(8, 12), ("qp", "ap")),
                ("vaug", 1, range(12, 16), ("qp", "ap")),
            ],
        }

        prev = None
        for i, (qc, t) in enumerate(streams):
            qsl = slice(qc * 512, (qc + 1) * 512)
            fill = list(filler.get(i, []))
            last = i == len(streams) - 1
            st = {
                "i": i,
                "qc": qc,
                "t": t,
                "qsl": qsl,
                "ets": [],
                "ctxs": [None, None],
                "ctag": ("qp", "ap") if last else ("cd", "cd"),
                # immediate (last stream) drains ascending, prev drains descending
                "start_j": 0 if last else NKC - 1,
                "stop_j": NKC - 1 if last else 0,
            }
            for kc in range(NKC):
                ksl = slice(kc * 128, (kc + 1) * 128)
                # drain first: always-ready PE work sits ahead of the scores
                # matmul that may wait on ACT freeing its PSUM buffer
                if prev is not None:
                    emit_drain_pair(prev, NKC - 1 - kc)
                sp = ps.tile([128, 1024], f32, tag="sp", name="sp", bufs=2)
                nc.tensor.matmul(
                    sp[:, 0:512],
                    lhsT=kk_sb[t][0:64, ksl],
                    rhs=q_sb[t][0:64, qsl],
                    start=True,
                    stop=True,
                )
                nc.tensor.matmul(
                    sp[:, 512:1024],
                    lhsT=kk_sb[t][64:128, ksl],
                    rhs=q_sb[t][64:128, qsl],
                    start=True,
                    stop=True,
                )
                et = etp.tile([128, 1024], mmdt, tag="et", name=f"et{kc}")
                nc.scalar.activation(et[:], sp[:], AF.Exp, scale=0.125)
                st["ets"].append(et)
                if last:
                    emit_drain_pair(st, kc)
                if fill and (kc % 2 == 1 or len(fill) >= NKC - kc):
                    run_filler(fill.pop(0))
            for item in fill:
                run_filler(item)
            if prev is not None:
                emit_finalize(prev)
            if last:
                emit_finalize(st)
                prev = None
            else:
                prev = st

    nc.compile()
    bacc.get_activation_tables = _orig_get_tables
    return nc


def kernel(hidden_states, attention_mask, Wq, bq, Wk, bk):
    global _compiled, LAST_RESULT
    hs = np.asarray(hidden_states, dtype=np.float32)
    am = np.asarray(attention_mask)
    Wq = np.asarray(Wq, dtype=np.float32)
    Wk = np.asarray(Wk, dtype=np.float32)
    bq = np.asarray(bq, dtype=np.float32)
    bk = np.asarray(bk, dtype=np.float32)

    if _compiled is None:
        _compiled = _build()
    nc = _compiled

    from concourse.bass_utils import run_bass_kernel_spmd

    import ml_dtypes

    def to_mmdt(x):
        return np.ascontiguousarray(np.asarray(x, np.float32).astype(ml_dtypes.bfloat16))

    idb = to_mmdt(np.tile(np.eye(64, dtype=np.float32), (2, 1)))
    in_maps = []
    for c in range(NCORES):
        b, g = c // HG, c % HG
        cols = slice(g * CPG, (g + 1) * CPG)
        bq_s = bq[cols].reshape(2, 128).T
        bk_s = bk[cols].reshape(2, 128).T
        smalls = np.concatenate(
            [bq_s, -bq_s, -(bq_s + bk_s), np.ones((128, 16), np.float32)], axis=1
        ).astype(np.float32)
        in_maps.append(
            {
                "ht": to_mmdt(hs[b].T),
                "wall": to_mmdt(
                    np.concatenate([Wq[:, cols], Wq[:, cols] + Wk[:, cols]], axis=1)
                ),
                "smalls": np.ascontiguousarray(smalls),
                "idb": idb,
            }
        )

    res = run_bass_kernel_spmd(nc, in_maps, list(range(NCORES)))
    LAST_RESULT = res

    outp = np.empty((B, S, H * DH), dtype=np.float32)
    for c in range(NCORES):
        b, g = c // HG, c % HG
        ctxT = res.results[c]["out"]  # [256, 2048] raw ctx sums (transposed)
        dn = res.results[c]["dens"]  # [16, 512]: row qc*4 + t*2 + rr
        den = np.empty((4, S), dtype=np.float32)
        for qc in range(NSC):
            for t in range(2):
                for rr in range(2):
                    den[t * 2 + rr, qc * 512 : (qc + 1) * 512] = dn[qc * 4 + t * 2 + rr]
        ctxT = ctxT.reshape(4, 64, S) / den[:, None, :]
        outp[b, :, g * CPG : (g + 1) * CPG] = ctxT.reshape(CPG, S).T

    # attention_mask==0 masks whole query rows -> uniform probs -> ctx row is
    # the mean of q over all key positions. Never triggers for all-ones masks.
    if (am == 0).any():
        for b in range(B):
            rows = np.nonzero(am[b] == 0)[0]
            if rows.size:
                q_full = hs[b] @ Wq + bq
                outp[b, rows, :] = q_full.mean(axis=0)
    return outp


# revision 23
# speedup vs baseline: 1.3900x; 1.0552x over previous
"""Trainium2 Bass kernel for BertSelfAttention variant (logsigmoid-fused QK attention).

Reference computation (B=2, S=2048, D=1024, H=16, dh=64):
    q = split_heads(hidden @ Wq + bq)
    k = split_heads(hidden @ Wk + bk)
    k1 = logsigmoid(q) + q + k ; k2 = logsigmoid(k1)
    scores = -(q @ k2^T) / 8 ; probs = softmax(scores) ; ctx = probs @ q

With kk := -k2 = ln(1 + e^{-q-k} + e^{-2q-k}) = ln(1 + e^{-q-k}(1 + e^{-q})),
scores == +(q . kk)/8.  Only two projection chains are needed (Wq and Wq+Wk,
host-fused): eu = e^{-(q+k)} from the fused chain, eq = e^{-q} from the q
chain (both straight from PSUM), kst = (eq+1)*eu in one DVE op, kk = Ln(kst+1).

Sharding: 8 cores = 2 (batch) x 4 (head groups of 4 heads / 256 cols of Wq,Wk).
Each core computes its [2048, 256] slice of the output; host reassembles.

Device layout is fully transposed so no matmul ever needs a transposed operand:
    qT, kkT [256(dout), 2048(s)]  from  ht = hidden[b].T  (host-side transpose)
    scoresT[kpos, q] = kkT_head^T @ qT_head   (row-tiled head pairs, K=64 at
        tile positions (0,0)/(64,0) -> the two matmuls run concurrently)
    expT = Exp(scoresT / 8)                   (one [128,1024] ACT op per chunk)
    ctx_aug[65, q] = sum_kpos v_aug[kpos,65]^T @ expT[kpos, q]
        v_aug = [v | 1] -> row 64 accumulates the softmax denominator.
Raw transposed ctx + denominators go back to DRAM; the host divides and
re-transposes while unsharding (no device-side finalize transposes).

Schedule: software pipeline over 8 t-major q-streams, per-chunk pipelined
startup (stream 0 begins after just the first projection chunk).  While ACT
streams the exps of stream i, the PE drains stream i-1's ctx accumulation
(drain pair emitted before the scores pair in each slot, so the PE never
head-blocks on a PSUM buffer with ready work behind it).  Projection chunks,
v transposes and the second-half Ln ride in the early streams' spare PE slots.
One 8-bank PSUM pool: sp(2x[128,1024]) qp ap cd(2x[65,512] ctx) - the tpv
transposes borrow cd (stream 0) / qp+ap (streams 2-3, after projections), and
the last stream's immediate ctx reuses qp/ap.  A single combined exp+ln
activation table avoids mid-kernel ACT_TABLE_LOAD switches.  Input DMA fans
out over four engine rings so the first projection chunk lands in ~6us.

Matmuls run in bf16; accumulators and the output stay fp32.
"""

import numpy as np

B, S, D = 2, 2048, 1024
H, DH = 16, 64
NCORES = 8
HG = 4  # head-group count (tensor parallel)
CPG = (H // HG) * DH  # cols per group = 256
NDT = D // 128  # 8 din tiles
NSC = S // 512  # 4 s-chunks (projection) == 4 q-chunks (attention)
NKC = S // 128  # 16 kpos chunks

_compiled = None
LAST_RESULT = None


def _build():
    from contextlib import ExitStack

    import concourse.bacc as bacc
    import concourse.mybir as mybir
    import concourse.tile as tile

    # Use the one activation table that holds BOTH Exp and Ln, so the kernel
    # does a single ACT_TABLE_LOAD instead of switching tables around each Ln.
    # act_func_set_id is the INDEX into act_info.json's act_func_sets, so keep
    # every entry (and its position) but empty the other sets so the
    # load-insertion pass can only ever pick the combined one.
    _orig_get_tables = bacc.get_activation_tables

    def _only_combined(arch):
        t = _orig_get_tables(arch)
        name = "natural_log_exp_and_others"
        if name not in t:
            return t
        return {k: (v if k == name else set()) for k, v in t.items()}

    bacc.get_activation_tables = _only_combined

    f32 = mybir.dt.float32
    mmdt = mybir.dt.bfloat16
    AF = mybir.ActivationFunctionType
    ALU = mybir.AluOpType

    nc = bacc.Bacc("TRN2", target_bir_lowering=False, debug=False)
    ht = nc.dram_tensor("ht", [D, S], mmdt, kind="ExternalInput").ap()
    # host-fused weight wall: [Wq | Wq+Wk] per row
    wall = nc.dram_tensor("wall", [D, 2 * CPG], mmdt, kind="ExternalInput").ap()
    # smalls cols: pbq[0:2] nbq[2:4] nbqk[4:6]
    smalls = nc.dram_tensor("smalls", [128, 6], f32, kind="ExternalInput").ap()
    # identity for the v transposes (I64 stacked twice) + a ones column
    idb = nc.dram_tensor("idb", [128, 65], mmdt, kind="ExternalInput").ap()
    out = nc.dram_tensor("out", [CPG, S], f32, kind="ExternalOutput").ap()
    # row t*4+qc = [h0 den(512q) | h1 den(512q)]
    dens = nc.dram_tensor("dens", [2 * NSC, 1024], f32, kind="ExternalOutput").ap()

    with tile.TileContext(nc) as tc, ExitStack() as ctx:
        const = ctx.enter_context(tc.tile_pool(name="const", bufs=1))
        big = ctx.enter_context(tc.tile_pool(name="big", bufs=1))
        sb = ctx.enter_context(tc.tile_pool(name="sb", bufs=4))
        # PSUM, exactly 8 banks: sp (2 bufs x [128,1024] = 4 banks),
        # qp, ap (1 bank each, projections), cd (2 bufs x 1 bank, ctx tiles).
        ps = ctx.enter_context(tc.tile_pool(name="ps", bufs=1, space="PSUM"))
        etp = ctx.enter_context(tc.tile_pool(name="etp", bufs=24))
        csp = ctx.enter_context(tc.tile_pool(name="csp", bufs=4))
        dtp = ctx.enter_context(tc.tile_pool(name="dtp", bufs=2))

        sm = const.tile([128, 6], f32, tag="smalls")
        nc.sync.dma_start(sm[:], smalls[:])
        pbq_t = sm[:, 0:2]
        nbq_t = sm[:, 2:4]
        nbqk_t = sm[:, 4:6]
        idb_t = const.tile([128, 65], mmdt, tag="idb")
        nc.sync.dma_start(idb_t[:], idb[:])
        ones_t = idb_t[:, 64:65]

        # Input DMA: weights + the sc0 hidden chunk gate the first projection
        # chunk, so interleave them across both HWDGE rings (SP + ACT, the
        # only engines that can initiate DMAs); together they run ~400GB/s.
        # Later hidden chunks follow in sc order so proj(t,sc) unblocks
        # progressively.
        rr_ring = [nc.sync, nc.scalar]
        ring_i = 0

        def ring():
            nonlocal ring_i
            ring_i += 1
            return rr_ring[ring_i % 2]

        wqs, was, hts = [], [], []
        for j in range(NDT):
            t_ = big.tile([128, S], mmdt, tag=f"ht{j}", name=f"hts{j}")
            hts.append(t_)
        for j in range(NDT):
            w = const.tile([128, 2 * CPG], mmdt, tag=f"w{j}", name=f"walls{j}")
            ring().dma_start(w[:], wall[j * 128 : (j + 1) * 128, :])
            wqs.append(w[:, 0:CPG])
            was.append(w[:, CPG : 2 * CPG])
            ring().dma_start(hts[j][:, 0:512], ht[j * 128 : (j + 1) * 128, 0:512])
        for sc in range(1, NSC):
            ssl = slice(sc * 512, (sc + 1) * 512)
            for j in range(NDT):
                nc.sync.dma_start(hts[j][:, ssl], ht[j * 128 : (j + 1) * 128, ssl])

        q_sb = [big.tile([128, S], mmdt, tag=f"q{t}", name=f"q{t}") for t in range(2)]
        kk_sb = [big.tile([128, S], mmdt, tag=f"kk{t}", name=f"kk{t}") for t in range(2)]
        kst = [big.tile([128, S], f32, tag=f"kst{t}", name=f"kst{t}") for t in range(2)]
        vaug = [
            big.tile([128, NKC * 64], mmdt, tag=f"v{h}", name=f"v{h}") for h in range(4)
        ]

        # ---------- emission helpers ----------
        def ps_tag_bufs(tag):
            return 2 if tag == "cd" else 1

        def emit_proj_chunk(t, sc, ln=True):
            """Two interleaved matmul chains for one [128, 512] chunk.

            Interleaving qp/ap per din tile means both chains finish one
            matmul after the last input DMA lands (startup critical path).
            """
            ssl = slice(sc * 512, (sc + 1) * 512)
            qp = ps.tile([128, 512], f32, tag="qp", name="qp", bufs=1)
            ap = ps.tile([128, 512], f32, tag="ap", name="ap", bufs=1)
            for j in range(NDT):
                nc.tensor.matmul(
                    qp[:],
                    lhsT=wqs[j][:, t * 128 : (t + 1) * 128],
                    rhs=hts[j][:, ssl],
                    start=(j == 0),
                    stop=(j == NDT - 1),
                )
                nc.tensor.matmul(
                    ap[:],
                    lhsT=was[j][:, t * 128 : (t + 1) * 128],
                    rhs=hts[j][:, ssl],
                    start=(j == 0),
                    stop=(j == NDT - 1),
                )
            eq = sb.tile([128, 512], f32, tag="eq")
            nc.scalar.activation(
                eq[:], qp[:], AF.Exp, bias=nbq_t[:, t : t + 1], scale=-1.0
            )
            nc.vector.tensor_scalar_add(q_sb[t][:, ssl], qp[:], pbq_t[:, t : t + 1])
            eu = sb.tile([128, 512], f32, tag="eu")
            nc.scalar.activation(
                eu[:], ap[:], AF.Exp, bias=nbqk_t[:, t : t + 1], scale=-1.0
            )
            # kst = (eq + 1) * eu = e^{-q-k} + e^{-2q-k}
            nc.vector.scalar_tensor_tensor(
                kst[t][:, ssl], eq[:], 1.0, eu[:], ALU.add, ALU.mult
            )
            if ln:
                nc.scalar.activation(
                    kk_sb[t][:, ssl], kst[t][:, ssl], AF.Ln, bias=1.0, scale=1.0
                )

        def emit_ln(t):
            nc.scalar.activation(kk_sb[t][:], kst[t][:], AF.Ln, bias=1.0, scale=1.0)

        def emit_vaug_chunk(t, j, tags):
            """PE-transpose one [64,128] q chunk per head of half t."""
            for rr in range(2):
                lh = 2 * t + rr
                hsl = slice(rr * 64, rr * 64 + 64)
                # sized [128,1024] bf16 = 2KB so the tag region matches the
                # [65,512]/[128,512] f32 tiles that share these banks
                tpv = ps.tile(
                    [128, 1024], mmdt, tag=tags[rr], name="tpv",
                    bufs=ps_tag_bufs(tags[rr]),
                )
                nc.tensor.transpose(
                    tpv[:, 0:64], q_sb[t][hsl, j * 128 : (j + 1) * 128], idb_t[hsl, 0:64]
                )
                nc.vector.tensor_copy(vaug[lh][:, j * 64 : j * 64 + 64], tpv[:, 0:64])

        def run_filler(item):
            if item[0] == "vaug":
                for j in item[2]:
                    emit_vaug_chunk(item[1], j, item[3])
            elif item[0] == "proj":
                emit_proj_chunk(item[1], item[2], ln=(item[1] == 0))
            elif item[0] == "ln":
                emit_ln(item[1])

        def emit_drain(st, rr, jj):
            """Ctx accumulation, one head, one kpos chunk.  h0 lands in its
            bank's partitions 0:64 (tile position (0,0)), h1 in the other
            bank's 64:128 ((0,64)) -> adjacent-emitted pairs of opposite
            heads run concurrently on separate PE column groups."""
            t = st["t"]
            if st["ctxs"][rr] is None:
                ctag = st["ctag"][rr]
                st["ctxs"][rr] = ps.tile(
                    [128, 512], f32, tag=ctag, name=f"ctx{rr}",
                    bufs=ps_tag_bufs(ctag),
                )
            nc.tensor.matmul(
                st["ctxs"][rr][rr * 64 : rr * 64 + 64, :],
                lhsT=vaug[2 * t + rr][:, jj * 64 : jj * 64 + 64],
                rhs=st["ets"][jj][:, rr * 512 : rr * 512 + 512],
                start=(jj == st["start_j"]),
                stop=(jj == st["stop_j"]),
            )

        def emit_den_push(st, tile):
            """Binary-counter tree merge of et tiles on DVE (all bf16 SBUF,
            so the adds run in the 2x performance mode).  After 16 pushes
            level 4 holds sum_kc et = the per-(head,q) denominators."""
            levels = st["dlv"]
            lvl, cur = 0, tile
            while len(levels) > lvl and levels[lvl] is not None:
                pend = levels[lvl]
                levels[lvl] = None
                nt = dtp.tile([128, 1024], mmdt, tag=f"dl{lvl}", name="dl", bufs=2)
                nc.vector.tensor_add(nt[:], pend[:], cur[:])
                cur = nt
                lvl += 1
            while len(levels) <= lvl:
                levels.append(None)
            levels[lvl] = cur

        def emit_den_mm_copy(st, tag):
            accb = st["dlv"][4]
            dps = ps.tile([128, 512], f32, tag=tag, name="denps", bufs=ps_tag_bufs(tag))
            nc.tensor.matmul(
                dps[0:1, :], lhsT=ones_t, rhs=accb[:, 0:512], start=True, stop=True
            )
            nc.tensor.matmul(
                dps[32:33, :], lhsT=ones_t, rhs=accb[:, 512:1024], start=True, stop=True
            )
            db = csp.tile([33, 512], f32, tag="densb", name="densb")
            nc.vector.tensor_copy(db[:], dps[0:33, :])
            r = st["t"] * NSC + st["qc"]
            nc.sync.dma_start(dens[r : r + 1, 0:512], db[0:1, :])
            nc.sync.dma_start(dens[r : r + 1, 512:1024], db[32:33, :])

        def emit_finalize(st):
            qsl = st["qsl"]
            t = st["t"]
            cs = csp.tile([128, 512], f32, tag="cs")
            nc.vector.tensor_copy(cs[0:64, :], st["ctxs"][0][0:64, :])
            nc.vector.tensor_copy(cs[64:128, :], st["ctxs"][1][64:128, :])
            nc.sync.dma_start(out[t * 128 : t * 128 + 128, qsl], cs[:])

        # ---------- schedule ----------
        emit_proj_chunk(0, 0)

        streams = [(qc, t) for t in range(2) for qc in range(NSC)]
        filler = {
            0: [
                ("proj", 0, 1),
                ("vaug", 0, range(0, 4), ("cd", "cd")),
                ("proj", 0, 2),
                ("vaug", 0, range(4, 8), ("cd", "cd")),
                ("proj", 0, 3),
                ("vaug", 0, range(8, 12), ("cd", "cd")),
                ("vaug", 0, range(12, 16), ("cd", "cd")),
                ("proj", 1, 0),
            ],
            1: [
                ("proj", 1, 1),
                ("proj", 1, 2),
                ("proj", 1, 3),
                ("ln", 1),
            ],
            2: [
                ("vaug", 1, range(0, 4), ("qp", "ap")),
                ("vaug", 1, range(4, 8), ("qp", "ap")),
            ],
            3: [
                ("vaug", 1, range(8, 12), ("qp", "ap")),
                ("vaug", 1, range(12, 16), ("qp", "ap")),
            ],
        }

        prev = None
        for i, (qc, t) in enumerate(streams):
            qsl = slice(qc * 512, (qc + 1) * 512)
            fill = list(filler.get(i, []))
            last = i == len(streams) - 1
            st = {
                "i": i,
                "qc": qc,
                "t": t,
                "qsl": qsl,
                "ets": [],
                "ctxs": [None, None],
                "dlv": [],
                "ctag": ("qp", "ap") if last else ("cd", "cd"),
                # immediate (last stream) drains ascending, prev drains descending
                "start_j": 0 if last else NKC - 1,
                "stop_j": NKC - 1 if last else 0,
            }
            for kc in range(NKC):
                ksl = slice(kc * 128, (kc + 1) * 128)
                # drains first: always-ready PE work sits ahead of the scores
                # matmul that may wait on ACT freeing its PSUM buffer.
                # Staggered opposite-head pairing so adjacent drain matmuls
                # sit on different PE column groups and run concurrently.
                if prev is not None:
                    emit_drain(prev, 0, NKC - 1 - kc)
                    if kc > 0:
                        emit_drain(prev, 1, NKC - kc)
                sp = ps.tile([128, 1024], f32, tag="sp", name="sp", bufs=2)
                nc.tensor.matmul(
                    sp[:, 0:512],
                    lhsT=kk_sb[t][0:64, ksl],
                    rhs=q_sb[t][0:64, qsl],
                    start=True,
                    stop=True,
                )
                nc.tensor.matmul(
                    sp[:, 512:1024],
                    lhsT=kk_sb[t][64:128, ksl],
                    rhs=q_sb[t][64:128, qsl],
                    start=True,
                    stop=True,
                )
                et = etp.tile([128, 1024], mmdt, tag="et", name=f"et{kc}")
                nc.scalar.activation(et[:], sp[:], AF.Exp, scale=0.125)
                st["ets"].append(et)
                emit_den_push(st, et)
                if last:
                    emit_drain(st, 0, kc)
                    if kc > 0:
                        emit_drain(st, 1, kc - 1)
                if fill and (kc % 2 == 1 or len(fill) >= NKC - kc):
                    run_filler(fill.pop(0))
            for item in fill:
                run_filler(item)
            if prev is not None:
                emit_drain(prev, 1, 0)
                emit_finalize(prev)
                emit_den_mm_copy(prev, "cd")
            if last:
                emit_drain(st, 1, NKC - 1)
                emit_finalize(st)
                emit_den_mm_copy(st, "qp")
                prev = None
            else:
                prev = st

    nc.compile()
    bacc.get_activation_tables = _orig_get_tables
    return nc


def kernel(hidden_states, attention_mask, Wq, bq, Wk, bk):
    global _compiled, LAST_RESULT
    hs = np.asarray(hidden_states, dtype=np.float32)
    am = np.asarray(attention_mask)
    Wq = np.asarray(Wq, dtype=np.float32)
    Wk = np.asarray(Wk, dtype=np.float32)
    bq = np.asarray(bq, dtype=np.float32)
    bk = np.asarray(bk, dtype=np.float32)

    if _compiled is None:
        _compiled = _build()
    nc = _compiled

    from concourse.bass_utils import run_bass_kernel_spmd

    import ml_dtypes

    def to_mmdt(x):
        return np.ascontiguousarray(np.asarray(x, np.float32).astype(ml_dtypes.bfloat16))

    idb = to_mmdt(
        np.concatenate(
            [np.tile(np.eye(64, dtype=np.float32), (2, 1)), np.ones((128, 1))], axis=1
        )
    )
    in_maps = []
    for c in range(NCORES):
        b, g = c // HG, c % HG
        cols = slice(g * CPG, (g + 1) * CPG)
        bq_s = bq[cols].reshape(2, 128).T
        bk_s = bk[cols].reshape(2, 128).T
        smalls = np.concatenate([bq_s, -bq_s, -(bq_s + bk_s)], axis=1).astype(
            np.float32
        )
        in_maps.append(
            {
                "ht": to_mmdt(hs[b].T),
                "wall": to_mmdt(
                    np.concatenate([Wq[:, cols], Wq[:, cols] + Wk[:, cols]], axis=1)
                ),
                "smalls": np.ascontiguousarray(smalls),
                "idb": idb,
            }
        )

    res = run_bass_kernel_spmd(nc, in_maps, list(range(NCORES)))
    LAST_RESULT = res

    outp = np.empty((B, S, H * DH), dtype=np.float32)
    for c in range(NCORES):
        b, g = c // HG, c % HG
        ctxT = res.results[c]["out"]  # [256, 2048] raw ctx sums (transposed)
        dn = res.results[c]["dens"]  # [8, 1024]: row t*4+qc = [h0 512q | h1 512q]
        den = np.empty((4, S), dtype=np.float32)
        for t in range(2):
            for qc in range(NSC):
                for rr in range(2):
                    den[t * 2 + rr, qc * 512 : (qc + 1) * 512] = dn[
                        t * NSC + qc, rr * 512 : (rr + 1) * 512
                    ]
        ctxT = ctxT.reshape(4, 64, S) / den[:, None, :]
        outp[b, :, g * CPG : (g + 1) * CPG] = ctxT.reshape(CPG, S).T

    # attention_mask==0 masks whole query rows -> uniform probs -> ctx row is
    # the mean of q over all key positions. Never triggers for all-ones masks.
    if (am == 0).any():
        for b in range(B):
            rows = np.nonzero(am[b] == 0)[0]
            if rows.size:
                q_full = hs[b] @ Wq + bq
                outp[b, rows, :] = q_full.mean(axis=0)
    return outp
